# revision 1
# baseline (speedup 1.0000x reference)
"""Trainium2 Bass kernel for nn_Encoder_Spatio (gnn_message_passing).

Math (validated against the reference to ~1e-6 rel):
    h1 = ELU(x @ fc1_w.T + b1)
    h2 = ELU(h1 @ fc2_w.T + b2)
    probs[b,i,j] = sq[b,i] + sk[b,j] + c
where BN (eval) and the Q/K projections + mlp2 halves fold on the host into
    vq, vk in R^256,  c scalar:    sq = h2 @ vq,  sk = h2 @ vk  (+consts).

Sharding: pure data-parallel over B (8 batches -> 8 cores). Each core runs the
identical program on its own batch; no collectives.

Device layout: activations are feature-major [feat(part), token(free)].
ELU(z) = relu(z) + min(exp(z) - 1, 0)   (exact, overflow-safe for our z range)
The [2048,2048] output is built from a broadcast tile T0[p,j] = sk[j] + c
(computed by a matmul whose stationary operand is vk replicated over 128
columns) plus a per-partition scalar sq chunk, then DMA'd out. Token slices
pipeline so output DMA overlaps compute (triangular emission).
"""

import sys

if "/opt/trn_rl_repo" not in sys.path:
    sys.path.insert(0, "/opt/trn_rl_repo")

import types

import numpy as np


def _ensure_axon_hooks():
    """concourse.bass_utils imports antenv.axon_hooks when tracing is
    requested; this image's antenv package lacks that submodule, which turns
    a skipped-trace fallback into a hard ImportError. Fill the hole with a
    None-hook stub (tracing degrades gracefully) if it's missing."""
    try:
        import antenv.axon_hooks  # noqa: F401
        return
    except ImportError:
        pass
    try:
        import antenv
    except ImportError:
        return
    mod = types.ModuleType("antenv.axon_hooks")
    mod._hook = None

    def set_axon_ntff_profile_hook(hook):
        mod._hook = hook

    def get_axon_ntff_profile_hook():
        return mod._hook

    mod.set_axon_ntff_profile_hook = set_axon_ntff_profile_hook
    mod.get_axon_ntff_profile_hook = get_axon_ntff_profile_hook
    sys.modules["antenv.axon_hooks"] = mod
    antenv.axon_hooks = mod


_ensure_axon_hooks()

from concourse import bass, tile, mybir
from concourse.bass_utils import run_bass_kernel_spmd

B, N, F = 8, 2048, 256      # batch, tokens, feature width (NIN == NHID == 256)
KC = F // 128               # feature chunks of 128 partitions
TS = 4                      # token slices
SW = N // TS                # slice width (512 = max fp32 matmul moving dim)
RBW = 128                   # output row-block width (partition dim)
NRB = N // RBW              # 16 row blocks
RB_PER_TS = SW // RBW       # row blocks per token slice

# Matmul operand dtype. float32r streams fp32 through the PE in one pass
# (4x faster than float32's hi/lo pass pair) but costs ~2.5e-4 relative
# error end-to-end (measured); float32 keeps the kernel exact (2.4e-7).
# The PE is not enough of a bottleneck here to justify the precision loss.
MM_DT = mybir.dt.float32

F32 = mybir.dt.float32
AF = mybir.ActivationFunctionType
ALU = mybir.AluOpType


def _split_multiwaits(nc):
    """This walrus build lowers at most one sync-wait per instruction on some
    instruction classes (the TileContext exit drain trips it). Hoist extra
    waits onto preceding single-wait drains on the same engine."""
    for f in nc.m.functions:
        for bb in f.blocks:
            insts = list(bb.instructions)
            out = []
            changed = False
            for inst in insts:
                si = inst.sync_info
                if si is not None and si.on_wait and len(si.on_wait) > 1:
                    waits = list(si.on_wait)
                    for k, w in enumerate(waits[:-1]):
                        d = mybir.InstDrain(name=f"{inst.name}-ws{k}")
                        d.engine = inst.engine
                        d.sync_info = mybir.SyncInfo(on_wait=[w], on_update=[])
                        out.append(d)
                    inst.sync_info = mybir.SyncInfo(
                        on_wait=[waits[-1]], on_update=list(si.on_update)
                    )
                    changed = True
                out.append(inst)
            if changed:
                bb.instructions = out


def _build_program():
    nc = bass.Bass(trn_type="TRN2")

    xt_d = nc.dram_tensor("xt", [F, N], F32, kind="ExternalInput")
    w1_d = nc.dram_tensor("w1t", [F, F], F32, kind="ExternalInput")
    b1_d = nc.dram_tensor("b1", [128, KC], F32, kind="ExternalInput")
    w2_d = nc.dram_tensor("w2t", [F, F], F32, kind="ExternalInput")
    b2_d = nc.dram_tensor("b2", [128, KC], F32, kind="ExternalInput")
    vkb_d = nc.dram_tensor("vkb", [F, 128], F32, kind="ExternalInput")
    vq_d = nc.dram_tensor("vq", [128, 2 * KC], F32, kind="ExternalInput")
    cb_d = nc.dram_tensor("cb", [128, 1], F32, kind="ExternalInput")
    out_d = nc.dram_tensor("out", [N, N], F32, kind="ExternalOutput")

    with tile.TileContext(nc) as tc:
        with (
            tc.tile_pool(name="wts", bufs=1) as wpool,
            tc.tile_pool(name="xin", bufs=4) as xpool,
            tc.tile_pool(name="eh", bufs=3) as epool,
            tc.tile_pool(name="rh", bufs=3) as rpool,
            tc.tile_pool(name="uh", bufs=3) as upool,
            tc.tile_pool(name="h1", bufs=3) as h1pool,
            tc.tile_pool(name="h2", bufs=3) as h2pool,
            tc.tile_pool(name="t0", bufs=1) as t0pool,
            tc.tile_pool(name="sqs", bufs=TS) as sqpool,
            tc.tile_pool(name="ob", bufs=10) as opool,
            tc.tile_pool(name="psmm", bufs=6, space="PSUM") as pspool,
            tc.tile_pool(name="pssq", bufs=2, space="PSUM") as sqps_pool,
        ):
            # ---- weights / constants (live for the whole kernel) ----
            w1 = [wpool.tile([128, F], MM_DT, tag=f"w1_{k}", name=f"w1_{k}") for k in range(KC)]
            w2 = [wpool.tile([128, F], MM_DT, tag=f"w2_{k}", name=f"w2_{k}") for k in range(KC)]
            vkb = [wpool.tile([128, 128], MM_DT, tag=f"vkb_{k}", name=f"vkb_{k}") for k in range(KC)]
            b1 = wpool.tile([128, KC], F32, tag="b1", name="b1")
            b2 = wpool.tile([128, KC], F32, tag="b2", name="b2")
            vq = wpool.tile([128, 2 * KC], MM_DT, tag="vq", name="vq")
            cb = wpool.tile([128, 1], F32, tag="cb", name="cb")

            def load_mm(dst, src_ap, stg_name):
                """Weight load on the sync HWDGE ring. For fp32r tiles,
                stage the f32 bytes and round with a DVE copy (the canonical
                "rounded to FP32r" producer the BIR verifier wants); fp32r
                IO tensors are rejected at runtime so the dram side stays
                f32."""
                if dst.dtype == F32:
                    nc.sync.dma_start(dst[:], src_ap)
                    return
                stg = wpool.tile(list(dst.shape), F32, tag=stg_name, name=stg_name)
                nc.sync.dma_start(stg[:], src_ap)
                nc.vector.tensor_copy(dst[:], stg[:])

            def load_x(s):
                """Input token-slice load on the scalar HWDGE ring (keeps
                the sync ring free for weights/output)."""
                xs = []
                for k in range(KC):
                    xt = xpool.tile([128, SW], F32, name="xt_t", tag="xt_t")
                    nc.scalar.dma_start(
                        xt[:], xt_d[k * 128:(k + 1) * 128, s * SW:(s + 1) * SW]
                    )
                    xs.append(xt)
                return xs

            # first input slices + first-layer weights lead the DMA queues
            # so the PE starts as early as possible
            xs = {0: load_x(0), 1: load_x(1)}
            for k in range(KC):
                load_mm(w1[k], w1_d[k * 128:(k + 1) * 128, :], f"w1s_{k}")
            nc.sync.dma_start(b1[:], b1_d[:])
            for k in range(KC):
                load_mm(w2[k], w2_d[k * 128:(k + 1) * 128, :], f"w2s_{k}")
            nc.sync.dma_start(b2[:], b2_d[:])
            for k in range(KC):
                load_mm(vkb[k], vkb_d[k * 128:(k + 1) * 128, :], f"vkbs_{k}")
            load_mm(vq, vq_d[:], "vqs")
            nc.sync.dma_start(cb[:], cb_d[:])

            # HAM warmup: dummy matmuls on memset tiles fill the PE during
            # the input-load window so the first real matmul runs at 2.4 GHz
            dmw = wpool.tile([128, 128], MM_DT, tag="dmw", name="dmw")
            dmx = wpool.tile([128, SW], MM_DT, tag="dmx", name="dmx")
            nc.gpsimd.memset(dmw[:], 0.0)
            nc.gpsimd.memset(dmx[:], 0.0)
            dps = pspool.tile([128, SW], F32, name="dps", tag="psmm")
            for w_i in range(3):
                nc.tensor.matmul(dps[:], dmw[:], dmx[:],
                                 start=(w_i == 0), stop=(w_i == 2))

            t0_full = t0pool.tile([128, N], F32, name="t0_full", tag="t0_full")
            t0 = []        # per-slice views into t0_full [128, SW]
            sq = []        # per-rowblock sq tiles [128, 1]
            h1s = {}       # slice -> KC h1 tiles
            h2s = {}       # slice -> KC h2 tiles
            n_out = 0

            def mlp_layer(w, bias, rhs_tiles, out_pool, tag):
                """One Linear+ELU layer for a token slice.
                rhs_tiles: KC tiles [128, SW] (feature-major input chunks).
                Returns KC tiles [128, SW] of ELU output."""
                outs = []
                for oc in range(KC):
                    ps = pspool.tile([128, SW], F32, name="ps_mm", tag="psmm")
                    for k in range(KC):
                        nc.tensor.matmul(
                            ps[:],
                            w[k][:, oc * 128:(oc + 1) * 128],
                            rhs_tiles[k][:],
                            start=(k == 0),
                            stop=(k == KC - 1),
                        )
                    bia = bias[:, oc:oc + 1]
                    e = epool.tile([128, SW], F32, name="e_t")
                    r = rpool.tile([128, SW], F32, name="r_t")
                    u = upool.tile([128, SW], F32, name="u_t")
                    h = out_pool.tile([128, SW], F32, tag=f"{tag}_{oc}", name=f"{tag}_{oc}")
                    # ELU(z) = relu(z) + min(exp(z) - 1, 0), z = ps + bias
                    nc.scalar.activation(e[:], ps[:], AF.Exp, bias=bia)
                    if oc == 0:
                        nc.scalar.activation(r[:], ps[:], AF.Relu, bias=bia)
                    else:
                        nc.vector.tensor_scalar(
                            r[:], ps[:], bia, 0.0, ALU.add, ALU.max
                        )
                    nc.vector.tensor_scalar(u[:], e[:], -1.0, 0.0, ALU.add, ALU.min)
                    nc.vector.tensor_tensor(h[:], r[:], u[:], ALU.add)
                    outs.append(h)
                return outs

            def emit_quad(q, s):
                """One output DMA covering row blocks 4q..4q+3 at column
                slice s: a [128, 4, SW] SBUF tile lands on the [512, SW]
                DRAM region via a (row-within-block, block, col) access
                pattern, so 4 row blocks ship in a single 1 MB transfer."""
                nonlocal n_out
                ot = opool.tile([128, RB_PER_TS, SW], F32, name="out_t", tag="out_t")
                for rb in range(RB_PER_TS):
                    i = q * RB_PER_TS + rb
                    r16 = n_out % 16
                    n_out += 1
                    dst = ot[:, rb, :]
                    if r16 in (0, 3, 6, 9, 12):
                        nc.scalar.activation(
                            dst, t0[s], AF.Identity, bias=sq[i]
                        )
                    else:
                        nc.vector.tensor_scalar(
                            dst, t0[s], sq[i], None, ALU.add
                        )
                dram = out_d[
                    q * RB_PER_TS * RBW:(q + 1) * RB_PER_TS * RBW,
                    s * SW:(s + 1) * SW,
                ].rearrange("(b p) c -> p b c", b=RB_PER_TS)
                # sync engine is nearly idle; keep output-DMA issue cost
                # off the busy scalar engine
                nc.sync.dma_start(dram, ot[:])

            def stage_A(s):
                h1s[s] = mlp_layer(w1, b1, xs.pop(s), h1pool, "h1")

            def stage_B(s):
                h2s[s] = mlp_layer(w2, b2, h1s.pop(s), h2pool, "h2")

            def stage_T0(s):
                """sk[j] + c broadcast to all partitions, then emit the
                column-slice-s quads of already-finished row blocks."""
                pst = pspool.tile([128, SW], F32, name="ps_t0", tag="psmm")
                for k in range(KC):
                    nc.tensor.matmul(
                        pst[:], vkb[k][:], h2s[s][k][:],
                        start=(k == 0), stop=(k == KC - 1),
                    )
                t0s = t0_full[:, s * SW:(s + 1) * SW]
                nc.scalar.activation(t0s, pst[:], AF.Identity, bias=cb[:, 0:1])
                t0.append(t0s)
                for q in range(s):
                    emit_quad(q, s)

            def stage_SQ(s):
                """sq for this slice's row blocks (fp32r ISA needs even
                moving/output dims, hence the zero-padded vq pairs), then
                emit this row-block quad at every ready column slice. All
                row blocks of the slice land in one PSUM tile so a single
                DVE copy moves them to SBUF."""
                qps = sqps_pool.tile([128, 2 * RB_PER_TS], F32, name="qps")
                for rb in range(RB_PER_TS):
                    for k in range(KC):
                        nc.tensor.matmul(
                            qps[:, 2 * rb:2 * rb + 2],
                            h2s[s][k][:, rb * RBW:(rb + 1) * RBW],
                            vq[:, 2 * k:2 * k + 2],
                            start=(k == 0),
                            stop=(k == KC - 1),
                        )
                sqt = sqpool.tile(
                    [128, 2 * RB_PER_TS], F32, tag=f"sq_{s}", name=f"sq_{s}"
                )
                nc.vector.tensor_copy(sqt[:], qps[:])
                for rb in range(RB_PER_TS):
                    sq.append(sqt[:, 2 * rb:2 * rb + 1])
                if s == TS - 1:
                    # last slice: the whole t0 row exists, so each new row
                    # block ships as one full-width op + one 1 MB DMA with
                    # 8 KB-contiguous descriptors
                    for rb in range(RB_PER_TS):
                        i = s * RB_PER_TS + rb
                        orow = opool.tile(
                            [128, N], F32, name="out_row", tag="out_t"
                        )
                        if rb % 2 == 0:
                            nc.scalar.activation(
                                orow[:], t0_full[:], AF.Identity, bias=sq[i]
                            )
                        else:
                            nc.vector.tensor_scalar(
                                orow[:], t0_full[:], sq[i], None, ALU.add
                            )
                        nc.sync.dma_start(
                            out_d[i * RBW:(i + 1) * RBW, :], orow[:]
                        )
                else:
                    for sp in range(s + 1):
                        emit_quad(s, sp)

            def stage_C(s):
                stage_T0(s)
                stage_SQ(s)

            # PE work order chosen so every stage's input ELU is already
            # done when the PE reaches it (near-zero PE bubbles) while the
            # broadcast/score stages (C) still land early enough to stream
            # output DMA during compute: A=L1, B=L2, C=T0+sq.
            stage_A(0)
            stage_A(1)
            stage_B(0)
            xs[2] = load_x(2)
            stage_A(2)
            stage_C(0)
            stage_B(1)
            xs[3] = load_x(3)
            stage_A(3)
            stage_C(1)
            stage_B(2)
            stage_B(3)
            stage_C(2)
            stage_C(3)

    _split_multiwaits(nc)
    return nc


_prog_cache = {}


def _get_program():
    if "nc" not in _prog_cache:
        _prog_cache["nc"] = _build_program()
    return _prog_cache["nc"]


def kernel(**inputs):
    inp = np.asarray(inputs["inputs"], np.float32)        # [B, N, F]
    fc1_w = np.asarray(inputs["fc1_w"], np.float64)
    fc1_b = np.asarray(inputs["fc1_b"], np.float64)
    fc2_w = np.asarray(inputs["fc2_w"], np.float64)
    fc2_b = np.asarray(inputs["fc2_b"], np.float64)
    bn_g = np.asarray(inputs["bn_g"], np.float64)
    bn_b = np.asarray(inputs["bn_b"], np.float64)
    bn_mean = np.asarray(inputs["bn_mean"], np.float64)
    bn_var = np.asarray(inputs["bn_var"], np.float64)
    wq_w = np.asarray(inputs["wq_w"], np.float64)
    wq_b = np.asarray(inputs["wq_b"], np.float64)
    wk_w = np.asarray(inputs["wk_w"], np.float64)
    wk_b = np.asarray(inputs["wk_b"], np.float64)
    mlp2_w = np.asarray(inputs["mlp2_w"], np.float64)
    mlp2_b = np.asarray(inputs["mlp2_b"], np.float64)

    # Fold BN (eval) into the Q/K projections, then both projections and the
    # mlp2 halves into two R^F vectors + one scalar (exact linear algebra).
    D = wq_w.shape[0]
    s = bn_g / np.sqrt(bn_var + 1e-5)
    t = bn_b - bn_mean * s
    wqf = wq_w * s[None, :]
    bqf = wq_b + wq_w @ t
    wkf = wk_w * s[None, :]
    bkf = wk_b + wk_w @ t
    wk_half, wq_half = mlp2_w[0, :D], mlp2_w[0, D:]
    vq = wqf.T @ wq_half
    vk = wkf.T @ wk_half
    c_total = float(bqf @ wq_half + bkf @ wk_half + mlp2_b[0])

    shared = {
        "w1t": np.ascontiguousarray(fc1_w.T, dtype=np.float32),
        "b1": np.ascontiguousarray(fc1_b.reshape(KC, 128).T, dtype=np.float32),
        "w2t": np.ascontiguousarray(fc2_w.T, dtype=np.float32),
        "b2": np.ascontiguousarray(fc2_b.reshape(KC, 128).T, dtype=np.float32),
        "vkb": np.ascontiguousarray(
            np.tile(vk[:, None], (1, 128)), dtype=np.float32
        ),
        "vq": np.ascontiguousarray(
            np.stack(
                [vq.reshape(KC, 128)[k // 2] if k % 2 == 0 else np.zeros(128)
                 for k in range(2 * KC)], axis=1
            ),
            dtype=np.float32,
        ),
        "cb": np.full((128, 1), c_total, np.float32),
    }
    in_maps = [
        {"xt": np.ascontiguousarray(inp[b].T), **shared} for b in range(B)
    ]

    nc = _get_program()
    res = run_bass_kernel_spmd(nc, in_maps, core_ids=list(range(B)))
    kernel.last_results = res
    return np.stack([res.results[b]["out"] for b in range(B)], axis=0)

